# revision 1
# baseline (speedup 1.0000x reference)
"""Trainium2 Bass kernel for nn_BarcodeSLayerEncoder.

Design (8 NeuronCores, pure data-parallel over batch):
  - Each core gets B/8 = 32 batches of both barcode tensors.
  - SLayer logits via ONE TensorE matmul per batch (K=120, N=512):
      logit[16g+e, p] = 2*s_x*c_x[e]*x - s_x*x^2 + 2*s_y*c_y[e]*y - s_y*y^2
                        - 100*maskflag(g,p)          (g = 4 h0-chunks + 4 h1-chunks)
    with the per-center constant -s*||c||^2 folded into the ScalarE Exp bias.
    The rhs rows (x, y, x^2, y^2, mask per group) are packed host-side into
    bf16 hi/lo residual pairs; the matmul streams 1 column/cycle (f32 would
    stream at 1/4 rate) while keeping ~1e-5 final accuracy.
  - exp on ScalarE (PSUM -> bf16 SBUF), point-sum on VectorE tensor_reduce.
  - Per-core SLayer features x [32feat, 32batch] -> AllGather (only collective) ->
    every core redundantly computes the tiny head (exact global BatchNorm stats).
  - Output per core: [128 out_features, 256 batch]; host takes core 0 and transposes.
"""

import sys

sys.path.insert(0, "/opt/trn_rl_repo")

import numpy as np

N_CORES = 8
B, P, E, D = 256, 2048, 16, 2
BL = B // N_CORES  # 32 local batches per core
HID, OUT = 128, 128
BN_EPS = 1e-5
CH = 512  # point chunk size (one PSUM bank of f32)
NCH = P // CH  # 4 chunks per homology
SB = 1  # one batch per supertile (120 hi/lo rhs rows at base 0)
NSUP = BL // SB  # 32 supertiles per core
MASK_W = -100.0

_CACHE = {}


def _build():
    from concourse import bacc, bass, mybir, tile

    f32 = mybir.dt.float32
    nc = bacc.Bacc("TRN2", target_bir_lowering=False, debug=False)

    # ---- kernel I/O ----
    bf16 = mybir.dt.bfloat16
    xin = nc.declare_dram_parameter("xpack", [NSUP, 128, CH], bf16, isOutput=False)
    w40_d = nc.declare_dram_parameter("w40", [128, 128], bf16, isOutput=False)
    ebias_d = nc.declare_dram_parameter("ebias", [128], f32, isOutput=False)
    r_d = nc.declare_dram_parameter("rmat", [128, 32], f32, isOutput=False)
    w1_d = nc.declare_dram_parameter("w1", [2 * E, HID], f32, isOutput=False)
    w2_d = nc.declare_dram_parameter("w2", [HID, OUT], f32, isOutput=False)
    g1_d = nc.declare_dram_parameter("gamma1", [HID], f32, isOutput=False)
    b1_d = nc.declare_dram_parameter("beta1", [HID], f32, isOutput=False)
    g2_d = nc.declare_dram_parameter("gamma2", [OUT], f32, isOutput=False)
    b2_d = nc.declare_dram_parameter("beta2", [OUT], f32, isOutput=False)
    ones_d = nc.declare_dram_parameter("ones128", [128], f32, isOutput=False)
    out_d = nc.declare_dram_parameter("out", [OUT, B], f32, isOutput=True)

    AF = mybir.ActivationFunctionType
    ALU = mybir.AluOpType
    groups = [list(range(N_CORES))]

    with tile.TileContext(nc) as tc:
        with (
            tc.tile_pool(name="consts", bufs=1) as cp,
            tc.tile_pool(name="xyt", bufs=8) as xyp,
            tc.tile_pool(name="resp", bufs=8) as rp,
            tc.tile_pool(name="small", bufs=1) as sp,
            tc.tile_pool(name="dram", bufs=1, space="DRAM") as dp,
        ):
            # prefetch the first supertiles before the const chain so the
            # first matmul isn't gated behind 12 serialized const DMAs
            pre = []
            for si in range(4):
                t = xyp.tile([128, CH], bf16, tag="xyt", name=f"pre{si}")
                (nc.sync, nc.gpsimd)[si % 2].dma_start(out=t[:], in_=xin[si])
                pre.append(t)

            # ---- load constants ----
            w40 = cp.tile([128, 128], bf16)
            nc.sync.dma_start(out=w40[:], in_=w40_d[:])
            ebias = cp.tile([128, 1], f32)
            nc.sync.dma_start(out=ebias[:], in_=ebias_d.rearrange("(m o) -> m o", o=1))
            rmat = cp.tile([128, 32], f32)
            nc.sync.dma_start(out=rmat[:], in_=r_d[:])
            w1 = cp.tile([2 * E, HID], f32)
            nc.sync.dma_start(out=w1[:], in_=w1_d[:])
            w2 = cp.tile([HID, OUT], f32)
            nc.sync.dma_start(out=w2[:], in_=w2_d[:])
            g1 = cp.tile([HID, 1], f32)
            nc.sync.dma_start(out=g1[:], in_=g1_d.rearrange("(m o) -> m o", o=1))
            b1 = cp.tile([HID, 1], f32)
            nc.sync.dma_start(out=b1[:], in_=b1_d.rearrange("(m o) -> m o", o=1))
            g2 = cp.tile([OUT, 1], f32)
            nc.sync.dma_start(out=g2[:], in_=g2_d.rearrange("(m o) -> m o", o=1))
            b2 = cp.tile([OUT, 1], f32)
            nc.sync.dma_start(out=b2[:], in_=b2_d.rearrange("(m o) -> m o", o=1))
            ones128 = cp.tile([128, 1], f32)
            nc.sync.dma_start(out=ones128[:], in_=ones_d.rearrange("(m o) -> m o", o=1))
            ones1 = cp.tile([1, 128], f32)
            nc.sync.dma_start(out=ones1[:], in_=ones_d.rearrange("(o m) -> o m", o=1))

            zero_t = cp.tile([128, 1], f32)
            nc.gpsimd.memset(zero_t[:], 0.0)
            eps_t = cp.tile([128, 1], f32)
            nc.gpsimd.memset(eps_t[:], BN_EPS)

            s_all = sp.tile([128, BL], f32)  # per-batch chunk-partial SLayer sums

            # ================= SLayer phase =================
            # host-packed supertiles: one batch per [128, CH] bf16 tile with
            # 120 rhs rows (data_hi/data_lo/data_hi paired against
            # w_hi/w_hi/w_lo for near-f32 logits at bf16 matmul rate)
            with tc.tile_pool(name="pslayer", bufs=7, space="PSUM") as pp:
                for si in range(NSUP):
                    if si < 4:
                        xyt = pre[si]
                    else:
                        xyt = xyp.tile([128, CH], bf16, tag="xyt")
                        eng = (nc.sync, nc.gpsimd)[si % 2]
                        eng.dma_start(out=xyt[:], in_=xin[si])
                    ps = pp.tile([128, CH], f32, tag="lg")
                    nc.tensor.matmul(
                        ps[:], w40[0:120, :], xyt[0:120, :],
                        start=True, stop=True,
                    )
                    resp = rp.tile([128, CH], bf16, tag="resp")
                    nc.scalar.activation(
                        resp[:], ps[:], AF.Exp, bias=ebias[:], scale=1.0,
                    )
                    nc.vector.tensor_reduce(
                        out=s_all[:, si : si + 1], in_=resp[:],
                        axis=mybir.AxisListType.X, op=ALU.add,
                    )

            # ================= head phase =================
            with tc.tile_pool(name="pst", bufs=1, space="PSUM") as pt:
                # x2[feat, local_batch] = R^T @ s_all
                x2_ps = pt.tile([32, BL], f32, tag="x2")
                nc.tensor.matmul(x2_ps[:], rmat[:], s_all[:])
                x2 = sp.tile([32, BL], f32)
                nc.vector.tensor_copy(x2[:], x2_ps[:])

                xb = dp.tile([32, BL], f32, name="xb")
                nc.sync.dma_start(out=xb[:], in_=x2[:])
                xg = dp.tile([N_CORES * 32, BL], f32, name="xg", addr_space="Shared")
                nc.gpsimd.collective_compute(
                    "AllGather",
                    ALU.bypass,
                    replica_groups=groups,
                    ins=[xb[:].opt()],
                    outs=[xg[:].opt()],
                )
                # gather -> xtf [32 feat, 256 batch] (batch = core-major)
                xtf = sp.tile([32, N_CORES, BL], f32)
                nc.sync.dma_start(
                    out=xtf[:], in_=xg[:].rearrange("(c f) b -> f c b", c=N_CORES)
                )
                xtf2 = xtf[:].rearrange("f c b -> f (c b)")

                # u = x @ W1  -> [128 hid, 256 batch]
                u_ps = pt.tile([HID, B], f32, tag="u")
                nc.tensor.matmul(u_ps[:], w1[:], xtf2[:])

                # BN1 stats (exact, global over the gathered batch)
                usum = sp.tile([HID, 1], f32)
                nc.vector.tensor_reduce(
                    out=usum[:], in_=u_ps[:], axis=mybir.AxisListType.X, op=ALU.add
                )
                usq_scr = rp.tile([HID, B], f32, tag="scr")
                usqs = sp.tile([HID, 1], f32)
                nc.scalar.activation(
                    usq_scr[:], u_ps[:], AF.Square, bias=zero_t[:], accum_out=usqs[:]
                )
                m1 = sp.tile([HID, 1], f32)
                nc.vector.tensor_scalar_mul(m1[:], usum[:], 1.0 / B)
                m1sq = sp.tile([HID, 1], f32)
                nc.vector.tensor_tensor(out=m1sq[:], in0=m1[:], in1=m1[:], op=ALU.mult)
                v1 = sp.tile([HID, 1], f32)
                nc.vector.tensor_scalar(
                    out=v1[:], in0=usqs[:], scalar1=1.0 / B, scalar2=None, op0=ALU.mult
                )
                nc.vector.tensor_tensor(out=v1[:], in0=v1[:], in1=m1sq[:], op=ALU.subtract)
                sd1 = sp.tile([HID, 1], f32)
                nc.scalar.activation(sd1[:], v1[:], AF.Sqrt, bias=eps_t[:])
                rs1 = sp.tile([HID, 1], f32)
                nc.vector.reciprocal(rs1[:], sd1[:])
                a1 = sp.tile([HID, 1], f32)
                nc.vector.tensor_tensor(out=a1[:], in0=rs1[:], in1=g1[:], op=ALU.mult)
                nb1 = sp.tile([HID, 1], f32)
                nc.vector.tensor_tensor(out=nb1[:], in0=m1[:], in1=a1[:], op=ALU.mult)
                nc.vector.tensor_tensor(out=nb1[:], in0=b1[:], in1=nb1[:], op=ALU.subtract)

                # h = relu(a1 * u + nb1)
                h = rp.tile([HID, B], f32, tag="scr")
                nc.scalar.activation(h[:], u_ps[:], AF.Relu, bias=nb1[:], scale=a1[:])

                # y = h @ W2 -> [128 out, 256 batch]
                y_ps = pt.tile([OUT, B], f32, tag="y")
                nc.tensor.matmul(y_ps[:], w2[:], h[:])

                ysum = sp.tile([OUT, 1], f32)
                nc.vector.tensor_reduce(
                    out=ysum[:], in_=y_ps[:], axis=mybir.AxisListType.X, op=ALU.add
                )
                ysq_scr = rp.tile([OUT, B], f32, tag="scr")
                ysqs = sp.tile([OUT, 1], f32)
                nc.scalar.activation(ysq_scr[:], y_ps[:], AF.Square, bias=zero_t[:], accum_out=ysqs[:])
                m2 = sp.tile([OUT, 1], f32)
                nc.vector.tensor_scalar_mul(m2[:], ysum[:], 1.0 / B)
                m2sq = sp.tile([OUT, 1], f32)
                nc.vector.tensor_tensor(out=m2sq[:], in0=m2[:], in1=m2[:], op=ALU.mult)
                v2 = sp.tile([OUT, 1], f32)
                nc.vector.tensor_scalar(
                    out=v2[:], in0=ysqs[:], scalar1=1.0 / B, scalar2=None, op0=ALU.mult
                )
                nc.vector.tensor_tensor(out=v2[:], in0=v2[:], in1=m2sq[:], op=ALU.subtract)
                sd2 = sp.tile([OUT, 1], f32)
                nc.scalar.activation(sd2[:], v2[:], AF.Sqrt, bias=eps_t[:])
                rs2 = sp.tile([OUT, 1], f32)
                nc.vector.reciprocal(rs2[:], sd2[:])
                a2 = sp.tile([OUT, 1], f32)
                nc.vector.tensor_tensor(out=a2[:], in0=rs2[:], in1=g2[:], op=ALU.mult)
                nb2 = sp.tile([OUT, 1], f32)
                nc.vector.tensor_tensor(out=nb2[:], in0=m2[:], in1=a2[:], op=ALU.mult)
                nc.vector.tensor_tensor(out=nb2[:], in0=b2[:], in1=nb2[:], op=ALU.subtract)

                # y_bn = a2*y + nb2 ; L2-normalize columns
                y_bn = rp.tile([OUT, B], f32, tag="scr2")
                nc.scalar.activation(y_bn[:], y_ps[:], AF.Identity, bias=nb2[:], scale=a2[:])
                ysq2 = rp.tile([OUT, B], f32, tag="scr")
                nc.scalar.activation(ysq2[:], y_bn[:], AF.Square, bias=zero_t[:])
                q_ps = pt.tile([1, B], f32, tag="q")
                nc.tensor.matmul(q_ps[:], ones128[:], ysq2[:])
                sdq = sp.tile([1, B], f32)
                nc.scalar.activation(sdq[:], q_ps[:], AF.Sqrt, bias=zero_t[0:1, :])
                rq = sp.tile([1, B], f32)
                nc.vector.reciprocal(rq[:], sdq[:])
                rnb_ps = pt.tile([128, B], f32, tag="rnb")
                nc.tensor.matmul(rnb_ps[:], ones1[:], rq[:])
                out_sb = rp.tile([OUT, B], f32, tag="scr3")
                nc.vector.tensor_tensor(
                    out=out_sb[:], in0=y_bn[:], in1=rnb_ps[:], op=ALU.mult
                )
                nc.sync.dma_start(out=out_d[:], in_=out_sb[:])

    nc.finalize()
    return nc


def _get_nc():
    if "nc" not in _CACHE:
        _CACHE["nc"] = _build()
    return _CACHE["nc"]


def _softplus(x):
    return np.logaddexp(0.0, x)


def _prep_weights(centers0, log_sharp0, centers1, log_sharp1):
    """Host-side packing of the tiny SLayer params into matmul weights."""
    import ml_dtypes

    s0 = _softplus(log_sharp0.astype(np.float64)) + 1e-6  # [E,2]
    s1 = _softplus(log_sharp1.astype(np.float64)) + 1e-6
    c0 = centers0.astype(np.float64)
    c1 = centers1.astype(np.float64)

    # 40 rhs rows per batch: j=0..7 x of group j (0..3 h0 chunks, 4..7 h1),
    # 8..15 y, 16..23 x^2, 24..31 y^2, 32..39 mask. outputs 16G+e.
    w1b = np.zeros((40, 128), np.float64)
    ebias = np.zeros(128, np.float64)
    for G in range(8):
        me = 16 * G + np.arange(E)
        sx, sy = (s0[:, 0], s0[:, 1]) if G < 4 else (s1[:, 0], s1[:, 1])
        cx, cy = (c0[:, 0], c0[:, 1]) if G < 4 else (c1[:, 0], c1[:, 1])
        w1b[G, me] = 2.0 * sx * cx
        w1b[8 + G, me] = 2.0 * sy * cy
        w1b[16 + G, me] = -sx
        w1b[24 + G, me] = -sy
        w1b[32 + G, me] = MASK_W
        ebias[me] = -(sx * cx**2 + sy * cy**2)

    # hi/lo residual rows: rows 0:40 pair w_hi with data_hi, rows 40:80 pair
    # w_hi with data_lo, rows 80:120 pair w_lo with data_hi
    import ml_dtypes as _mld
    w_hi = w1b.astype(_mld.bfloat16).astype(np.float64)
    w_lo = w1b - w_hi
    w40 = np.zeros((128, 128), np.float64)
    w40[0:40] = w_hi
    w40[40:80] = w_hi
    w40[80:120] = w_lo

    rmat = np.zeros((128, 32), np.float64)
    for g in range(4):
        for e in range(E):
            rmat[16 * g + e, e] = 1.0
            rmat[64 + 16 * g + e, 16 + e] = 1.0

    return (
        w40.astype(ml_dtypes.bfloat16),
        ebias.astype(np.float32),
        rmat.astype(np.float32),
    )


def _pack_core(bc0, bc1, c0, c1):
    """Pack one core's barcodes into [NSUP, 128, CH] bf16 rhs supertiles."""
    import ml_dtypes

    BLc = bc0.shape[0]
    blk = np.zeros((BLc, 128, CH), np.float32)  # per-batch rows at base 0
    pidx = np.arange(P, dtype=np.float32).reshape(NCH, CH)  # global point index
    for h, (bc, cnt) in enumerate(((bc0, c0), (bc1, c1))):
        xy = bc.reshape(BLc, NCH, CH, 2)
        g0 = 4 * h
        blk[:, g0 : g0 + 4] = xy[..., 0]
        blk[:, 8 + g0 : 12 + g0] = xy[..., 1]
        blk[:, 16 + g0 : 20 + g0] = xy[..., 0] ** 2
        blk[:, 24 + g0 : 28 + g0] = xy[..., 1] ** 2
        mask = (pidx[None] >= cnt[:, None, None]).astype(np.float32)
        blk[:, 32 + g0 : 36 + g0] = mask
    # one batch per supertile: rows 0:40 data_hi, 40:80 data_lo, 80:120 data_hi
    hi = blk[:, 0:40].astype(ml_dtypes.bfloat16)
    lo = (blk[:, 0:40] - hi.astype(np.float32)).astype(ml_dtypes.bfloat16)
    sup = np.zeros((NSUP, 128, CH), ml_dtypes.bfloat16)
    sup[:, 0:40] = hi
    sup[:, 40:80] = lo
    sup[:, 80:120] = hi
    return sup


def kernel(
    barcode_h0,
    barcode_h0_count,
    barcode_h1,
    barcode_h1_count,
    centers0,
    log_sharp0,
    centers1,
    log_sharp1,
    W1,
    gamma1,
    beta1,
    W2,
    gamma2,
    beta2,
):
    from concourse.bass_utils import run_bass_kernel_spmd

    nc = _get_nc()
    w40, ebias, rmat = _prep_weights(centers0, log_sharp0, centers1, log_sharp1)
    ones128 = np.ones(128, np.float32)

    bc0 = np.ascontiguousarray(barcode_h0, dtype=np.float32)
    bc1 = np.ascontiguousarray(barcode_h1, dtype=np.float32)
    c0 = np.asarray(barcode_h0_count).astype(np.float32)
    c1 = np.asarray(barcode_h1_count).astype(np.float32)

    in_maps = []
    for c in range(N_CORES):
        sl = slice(c * BL, (c + 1) * BL)
        in_maps.append(
            {
                "xpack": _pack_core(bc0[sl], bc1[sl], c0[sl], c1[sl]),
                "w40": w40,
                "ebias": ebias,
                "rmat": rmat,
                "w1": np.ascontiguousarray(W1, np.float32),
                "w2": np.ascontiguousarray(W2, np.float32),
                "gamma1": np.ascontiguousarray(gamma1, np.float32),
                "beta1": np.ascontiguousarray(beta1, np.float32),
                "gamma2": np.ascontiguousarray(gamma2, np.float32),
                "beta2": np.ascontiguousarray(beta2, np.float32),
                "ones128": ones128,
            }
        )

    _CACHE["in_maps"] = in_maps
    res = run_bass_kernel_spmd(nc, in_maps, core_ids=list(range(N_CORES)))
    out = res.results[0]["out"]  # [OUT, B]
    return np.ascontiguousarray(out.T).astype(np.float32)



# revision 2
# speedup vs baseline: 1.2184x; 1.2184x over previous
"""Trainium2 Bass kernel for nn_BarcodeSLayerEncoder.

Design (8 NeuronCores, pure data-parallel over batch):
  - Each core gets B/8 = 32 batches of both barcode tensors.
  - SLayer logits via TensorE matmuls (K=120, 512 cols per batch):
      logit[16g+e, p] = 2*s_x*c_x[e]*x - s_x*x^2 + 2*s_y*c_y[e]*y - s_y*y^2
                        - 100*maskflag(g,p)          (g = 4 h0-chunks + 4 h1-chunks)
    with the per-center constant -s*||c||^2 folded into the ScalarE Exp bias.
    Supertiles are processed in blocks of up to 4 batches so one wide Exp
    activation covers [128, 2048] (amortizes the ACT access latency).
  - Point sums via a bf16 TensorTensor halving tree on DVE (4x perf mode)
    finished by one grouped tensor_reduce -> s_all bf16.
  - Features x2 [32,32] bf16 -> AllGather (bf16 halves the collective bytes) ->
    every core redundantly computes the tiny head with exact global BN stats
    (bn_stats/bn_aggr for mean+var in two DVE ops).
  - Act tables: a dummy Exp at t=0 preloads the exp table under the DMA
    prologue; a dummy Sqrt right after the last Exp switches to the sqrt
    table under the collective. Dummy matmuls keep the PE p-state at full
    clock through the collective so the head matmuls run at 2.4 GHz.
"""

import sys

sys.path.insert(0, "/opt/trn_rl_repo")

import numpy as np

N_CORES = 8
B, P, E, D = 256, 2048, 16, 2
BL = B // N_CORES  # 32 local batches per core
HID, OUT = 128, 128
BN_EPS = 1e-5
CH = 512  # point chunk size (one PSUM bank of f32)
NCH = P // CH  # 4 chunks per homology
NSUP = BL  # one batch per supertile
MASK_W = -100.0
BLOCK_SIZES = [1, 3, 4, 4, 4, 4, 4, 4, 4]  # supertiles per exp block
NDUMMY = 88  # PE keep-warm matmuls spanning the collective window

# wpack (bf16) column layout
W40_C = 0  # [0:128] w40 hi/lo slayer weights
RMAT_C = 128  # [128:160] rmat
ONES_C = 160  # [160] ones column
ONES1_C = 161  # row 0, [161:289] ones row
W1_C = 289  # rows 0:32, [289:417] W1
W2_C = 417  # [417:545] W2
WPACK_W = 545
# cpack (f32) column layout: ebias, g1, b1, g2, b2
CPACK_W = 5

_CACHE = {}


def _build():
    from concourse import bacc, bass, mybir, tile

    f32 = mybir.dt.float32
    bf16 = mybir.dt.bfloat16
    nc = bacc.Bacc("TRN2", target_bir_lowering=False, debug=False)

    # ---- kernel I/O ----
    xin = nc.declare_dram_parameter("xpack", [128, NSUP * CH], bf16, isOutput=False)
    wpack_d = nc.declare_dram_parameter("wpack", [128, WPACK_W], bf16, isOutput=False)
    cpack_d = nc.declare_dram_parameter("cpack", [128, CPACK_W], f32, isOutput=False)
    out_d = nc.declare_dram_parameter("out", [OUT, B], f32, isOutput=True)

    AF = mybir.ActivationFunctionType
    ALU = mybir.AluOpType
    AX = mybir.AxisListType
    groups = [list(range(N_CORES))]

    with tile.TileContext(nc) as tc:
        with (
            tc.tile_pool(name="consts", bufs=1) as cp,
            tc.tile_pool(name="xyt", bufs=3) as xyp,
            tc.tile_pool(name="resp", bufs=2) as rp,
            tc.tile_pool(name="small", bufs=1) as sp,
            tc.tile_pool(name="dram", bufs=1, space="DRAM") as dp,
        ):
            zero_t = cp.tile([128, 1], f32)
            nc.gpsimd.memset(zero_t[:], 0.0)
            eps_t = cp.tile([128, 1], f32)
            nc.gpsimd.memset(eps_t[:], BN_EPS)
            zero_s = cp.tile([128, 512], bf16)
            nc.gpsimd.memset(zero_s[:], 0.0)

            # dummy exp: preloads the exp act table under the DMA prologue
            d0 = sp.tile([128, 1], bf16)
            nc.scalar.activation(d0[:], zero_t[:], AF.Exp, bias=zero_t[:], scale=1.0)

            # ---- batched constant loads ----
            wpack = cp.tile([128, WPACK_W], bf16)
            nc.sync.dma_start(out=wpack[:], in_=wpack_d[:])
            cpack = cp.tile([128, CPACK_W], f32)
            nc.sync.dma_start(out=cpack[:], in_=cpack_d[:])
            ebias = cpack[:, 0:1]
            g1 = cpack[:, 1:2]
            b1 = cpack[:, 2:3]
            g2 = cpack[:, 3:4]
            b2 = cpack[:, 4:5]
            w40 = wpack[0:120, W40_C : W40_C + 128]
            rmat = wpack[:, RMAT_C : RMAT_C + 32]
            ones128 = wpack[:, ONES_C : ONES_C + 1]
            ones1 = wpack[0:1, ONES1_C : ONES1_C + 128]
            w1 = wpack[0 : 2 * E, W1_C : W1_C + HID]
            w2 = wpack[:, W2_C : W2_C + OUT]

            s_all = sp.tile([128, BL], bf16)  # per-batch chunk-partial SLayer sums

            # ================= SLayer phase =================
            with tc.tile_pool(name="pslayer", bufs=2, space="PSUM") as pp:
                offs = 0
                for bi, n in enumerate(BLOCK_SIZES):
                    w = n * CH
                    xyt = xyp.tile([128, 4 * CH], bf16, tag="xyt", name=f"xyt{bi}")
                    eng = (nc.sync, nc.gpsimd)[bi % 2]
                    eng.dma_start(
                        out=xyt[:, 0:w], in_=xin[:, offs * CH : offs * CH + w]
                    )
                    ps = pp.tile([128, 4 * CH], f32, tag="lg")
                    for j in range(n):
                        nc.tensor.matmul(
                            ps[:, j * CH : (j + 1) * CH],
                            w40,
                            xyt[0:120, j * CH : (j + 1) * CH],
                            start=True,
                            stop=True,
                        )
                    resp = rp.tile([128, 4 * CH], bf16, tag="resp")
                    nc.scalar.activation(
                        resp[:, 0:w], ps[:, 0:w], AF.Exp, bias=ebias, scale=1.0
                    )
                    # bf16 halving tree on DVE (4x mode), then grouped reduce
                    r3 = resp[:, 0:w].rearrange("p (s c) -> p s c", s=n)
                    t1 = rp.tile([128, 4, 256], bf16, tag="t1")
                    nc.vector.tensor_tensor(
                        out=t1[:, 0:n, :], in0=r3[:, :, 0:256], in1=r3[:, :, 256:512],
                        op=ALU.add,
                    )
                    t2 = rp.tile([128, 4, 128], bf16, tag="t2")
                    nc.vector.tensor_tensor(
                        out=t2[:, 0:n, :], in0=t1[:, 0:n, 0:128], in1=t1[:, 0:n, 128:256],
                        op=ALU.add,
                    )
                    t3 = rp.tile([128, 4, 64], bf16, tag="t3")
                    nc.vector.tensor_tensor(
                        out=t3[:, 0:n, :], in0=t2[:, 0:n, 0:64], in1=t2[:, 0:n, 64:128],
                        op=ALU.add,
                    )
                    with nc.allow_low_precision("bf16 chunk sums, tol 2e-2"):
                        nc.vector.tensor_reduce(
                            out=s_all[:, offs : offs + n], in_=t3[:, 0:n, :],
                            axis=AX.X, op=ALU.add,
                        )
                    offs += n

            # dummy sqrt: switches the act table under the collective window
            dsq = sp.tile([128, 1], f32)
            nc.scalar.activation(dsq[:], eps_t[:], AF.Sqrt, bias=zero_t[:], scale=1.0)

            # ================= head phase =================
            with tc.tile_pool(name="pst", bufs=1, space="PSUM") as pt:
                # x2[feat, local_batch] = R^T @ s_all
                x2_ps = pt.tile([32, BL], f32, tag="x2")
                nc.tensor.matmul(x2_ps[:], rmat, s_all[:])
                x2 = sp.tile([32, BL], bf16)
                nc.vector.tensor_copy(x2[:], x2_ps[:])

                xb = dp.tile([32, BL], bf16, name="xb")
                nc.sync.dma_start(out=xb[:], in_=x2[:])
                xg = dp.tile([N_CORES * 32, BL], bf16, name="xg", addr_space="Shared")
                nc.gpsimd.collective_compute(
                    "AllGather",
                    ALU.bypass,
                    replica_groups=groups,
                    ins=[xb[:].opt()],
                    outs=[xg[:].opt()],
                )

                # keep the PE p-state hot through the collective
                dmt = pt.tile([128, 512], f32, tag="dummy")
                for _ in range(NDUMMY):
                    nc.tensor.matmul(dmt[:], zero_s[:, 0:128], zero_s[:])

                # gather -> xtf [32 feat, 256 batch] (batch = core-major)
                xtf = sp.tile([32, N_CORES, BL], bf16)
                nc.sync.dma_start(
                    out=xtf[:], in_=xg[:].rearrange("(c f) b -> f c b", c=N_CORES)
                )
                xtf2 = xtf[:].rearrange("f c b -> f (c b)")

                # u = x @ W1  -> [128 hid, 256 batch]
                u_ps = pt.tile([HID, B], f32, tag="u")
                nc.tensor.matmul(u_ps[:], w1, xtf2)

                # BN1 stats (exact, global over the gathered batch)
                st1 = sp.tile([HID, 6], f32)
                nc.vector.bn_stats(st1[:], u_ps[:])
                mv1 = sp.tile([HID, 2], f32)
                nc.vector.bn_aggr(mv1[:], st1[:])
                sd1 = sp.tile([HID, 1], f32)
                nc.scalar.activation(sd1[:], mv1[:, 1:2], AF.Sqrt, bias=eps_t[:])
                rs1 = sp.tile([HID, 1], f32)
                nc.vector.reciprocal(rs1[:], sd1[:])
                a1 = sp.tile([HID, 1], f32)
                nc.vector.tensor_tensor(out=a1[:], in0=rs1[:], in1=g1, op=ALU.mult)
                nb1 = sp.tile([HID, 1], f32)
                nc.vector.tensor_tensor(out=nb1[:], in0=mv1[:, 0:1], in1=a1[:], op=ALU.mult)
                nc.vector.tensor_tensor(out=nb1[:], in0=b1, in1=nb1[:], op=ALU.subtract)

                # h = relu(a1 * u + nb1)
                h = rp.tile([HID, B], bf16, tag="h")
                nc.scalar.activation(h[:], u_ps[:], AF.Relu, bias=nb1[:], scale=a1[:])

                # y = h @ W2 -> [128 out, 256 batch]
                y_ps = pt.tile([OUT, B], f32, tag="y")
                nc.tensor.matmul(y_ps[:], w2, h[:])

                st2 = sp.tile([OUT, 6], f32)
                nc.vector.bn_stats(st2[:], y_ps[:])
                mv2 = sp.tile([OUT, 2], f32)
                nc.vector.bn_aggr(mv2[:], st2[:])
                sd2 = sp.tile([OUT, 1], f32)
                nc.scalar.activation(sd2[:], mv2[:, 1:2], AF.Sqrt, bias=eps_t[:])
                rs2 = sp.tile([OUT, 1], f32)
                nc.vector.reciprocal(rs2[:], sd2[:])
                a2 = sp.tile([OUT, 1], f32)
                nc.vector.tensor_tensor(out=a2[:], in0=rs2[:], in1=g2, op=ALU.mult)
                nb2 = sp.tile([OUT, 1], f32)
                nc.vector.tensor_tensor(out=nb2[:], in0=mv2[:, 0:1], in1=a2[:], op=ALU.mult)
                nc.vector.tensor_tensor(out=nb2[:], in0=b2, in1=nb2[:], op=ALU.subtract)

                # y_bn = a2*y + nb2 ; L2-normalize columns
                y_bn = rp.tile([OUT, B], bf16, tag="ybn")
                nc.scalar.activation(y_bn[:], y_ps[:], AF.Identity, bias=nb2[:], scale=a2[:])
                ysq = rp.tile([OUT, B], bf16, tag="ysq")
                nc.scalar.activation(ysq[:], y_bn[:], AF.Square, bias=zero_t[:])
                q_ps = pt.tile([1, B], f32, tag="q")
                nc.tensor.matmul(q_ps[:], ones128, ysq[:])
                sdq = sp.tile([1, B], f32)
                nc.scalar.activation(sdq[:], q_ps[:], AF.Sqrt, bias=zero_t[0:1, :])
                rq = sp.tile([1, B], bf16)
                with nc.allow_low_precision("norm scale bf16, tol 2e-2"):
                    nc.vector.reciprocal(rq[:], sdq[:])
                rnb_ps = pt.tile([128, B], f32, tag="rnb")
                nc.tensor.matmul(rnb_ps[:], ones1, rq[:])
                out_sb = rp.tile([OUT, B], f32, tag="osb")
                nc.vector.tensor_tensor(
                    out=out_sb[:], in0=y_bn[:], in1=rnb_ps[:], op=ALU.mult
                )
                nc.sync.dma_start(out=out_d[:], in_=out_sb[:])

    nc.finalize()
    return nc


def _get_nc():
    if "nc" not in _CACHE:
        _CACHE["nc"] = _build()
    return _CACHE["nc"]


def _softplus(x):
    return np.logaddexp(0.0, x)


def _prep_weights(centers0, log_sharp0, centers1, log_sharp1, W1, W2,
                  gamma1, beta1, gamma2, beta2):
    """Host-side packing of the tiny SLayer/head params into two tiles."""
    import ml_dtypes

    s0 = _softplus(log_sharp0.astype(np.float64)) + 1e-6  # [E,2]
    s1 = _softplus(log_sharp1.astype(np.float64)) + 1e-6
    c0 = centers0.astype(np.float64)
    c1 = centers1.astype(np.float64)

    # 40 rhs rows per batch: j=0..7 x of group j (0..3 h0 chunks, 4..7 h1),
    # 8..15 y, 16..23 x^2, 24..31 y^2, 32..39 mask. outputs 16G+e.
    w1b = np.zeros((40, 128), np.float64)
    ebias = np.zeros(128, np.float64)
    for G in range(8):
        me = 16 * G + np.arange(E)
        sx, sy = (s0[:, 0], s0[:, 1]) if G < 4 else (s1[:, 0], s1[:, 1])
        cx, cy = (c0[:, 0], c0[:, 1]) if G < 4 else (c1[:, 0], c1[:, 1])
        w1b[G, me] = 2.0 * sx * cx
        w1b[8 + G, me] = 2.0 * sy * cy
        w1b[16 + G, me] = -sx
        w1b[24 + G, me] = -sy
        w1b[32 + G, me] = MASK_W
        ebias[me] = -(sx * cx**2 + sy * cy**2)

    # hi/lo residual rows: rows 0:40 pair w_hi with data_hi, rows 40:80 pair
    # w_hi with data_lo, rows 80:120 pair w_lo with data_hi
    w_hi = w1b.astype(ml_dtypes.bfloat16).astype(np.float64)
    w_lo = w1b - w_hi
    w40 = np.zeros((128, 128), np.float64)
    w40[0:40] = w_hi
    w40[40:80] = w_hi
    w40[80:120] = w_lo

    rmat = np.zeros((128, 32), np.float64)
    for g in range(4):
        for e in range(E):
            rmat[16 * g + e, e] = 1.0
            rmat[64 + 16 * g + e, 16 + e] = 1.0

    wpack = np.zeros((128, WPACK_W), np.float64)
    wpack[:, W40_C : W40_C + 128] = w40
    wpack[:, RMAT_C : RMAT_C + 32] = rmat
    wpack[:, ONES_C] = 1.0
    wpack[0, ONES1_C : ONES1_C + 128] = 1.0
    wpack[0 : 2 * E, W1_C : W1_C + HID] = W1.astype(np.float64)
    wpack[:, W2_C : W2_C + OUT] = W2.astype(np.float64)

    cpack = np.zeros((128, CPACK_W), np.float32)
    cpack[:, 0] = ebias.astype(np.float32)
    cpack[:, 1] = np.asarray(gamma1, np.float32)
    cpack[:, 2] = np.asarray(beta1, np.float32)
    cpack[:, 3] = np.asarray(gamma2, np.float32)
    cpack[:, 4] = np.asarray(beta2, np.float32)

    return wpack.astype(ml_dtypes.bfloat16), cpack


def _pack_core(bc0, bc1, c0, c1):
    """Pack one core's barcodes into a [128, NSUP*CH] bf16 rhs supertile row."""
    import ml_dtypes

    BLc = bc0.shape[0]
    blk = np.zeros((BLc, 128, CH), np.float32)  # per-batch rows at base 0
    pidx = np.arange(P, dtype=np.float32).reshape(NCH, CH)  # global point index
    for h, (bc, cnt) in enumerate(((bc0, c0), (bc1, c1))):
        xy = bc.reshape(BLc, NCH, CH, 2)
        g0 = 4 * h
        blk[:, g0 : g0 + 4] = xy[..., 0]
        blk[:, 8 + g0 : 12 + g0] = xy[..., 1]
        blk[:, 16 + g0 : 20 + g0] = xy[..., 0] ** 2
        blk[:, 24 + g0 : 28 + g0] = xy[..., 1] ** 2
        mask = (pidx[None] >= cnt[:, None, None]).astype(np.float32)
        blk[:, 32 + g0 : 36 + g0] = mask
    # one batch per supertile: rows 0:40 data_hi, 40:80 data_lo, 80:120 data_hi
    hi = blk[:, 0:40].astype(ml_dtypes.bfloat16)
    lo = (blk[:, 0:40] - hi.astype(np.float32)).astype(ml_dtypes.bfloat16)
    sup = np.zeros((BLc, 128, CH), ml_dtypes.bfloat16)
    sup[:, 0:40] = hi
    sup[:, 40:80] = lo
    sup[:, 80:120] = hi
    # supertile-major along the free dim: [128, NSUP*CH]
    return np.ascontiguousarray(sup.transpose(1, 0, 2).reshape(128, BLc * CH))


def kernel(
    barcode_h0,
    barcode_h0_count,
    barcode_h1,
    barcode_h1_count,
    centers0,
    log_sharp0,
    centers1,
    log_sharp1,
    W1,
    gamma1,
    beta1,
    W2,
    gamma2,
    beta2,
):
    from concourse.bass_utils import run_bass_kernel_spmd

    nc = _get_nc()
    wpack, cpack = _prep_weights(
        centers0, log_sharp0, centers1, log_sharp1, W1, W2,
        gamma1, beta1, gamma2, beta2,
    )

    bc0 = np.ascontiguousarray(barcode_h0, dtype=np.float32)
    bc1 = np.ascontiguousarray(barcode_h1, dtype=np.float32)
    c0 = np.asarray(barcode_h0_count).astype(np.float32)
    c1 = np.asarray(barcode_h1_count).astype(np.float32)

    in_maps = []
    for c in range(N_CORES):
        sl = slice(c * BL, (c + 1) * BL)
        in_maps.append(
            {
                "xpack": _pack_core(bc0[sl], bc1[sl], c0[sl], c1[sl]),
                "wpack": wpack,
                "cpack": cpack,
            }
        )

    _CACHE["in_maps"] = in_maps
    res = run_bass_kernel_spmd(nc, in_maps, core_ids=list(range(N_CORES)))
    out = res.results[0]["out"]  # [OUT, B]
    return np.ascontiguousarray(out.T).astype(np.float32)
